# revision 17
# baseline (speedup 1.0000x reference)
"""PaiNN-style GNN message passing on 8 trn2 NeuronCores (Bass/Tile).

Strategy: atoms sharded by molecule blocks (batch is sorted); each edge is
owned by the core that owns its row (scatter target). Gathers of neighbor
state s/v read a replicated fp16 table (8*NPAD, 4F) = [s|vx|vy|vz] in DRAM,
rebuilt each layer with an AllGather. Segment-sums are PE matmuls against
host-precomputed one-hot B blocks (edges sorted by 128-atom row windows,
per-window tile counts padded to the max across cores so all 8 cores run one
identical SPMD program).
"""
import numpy as np

F = 128          # feature dim (asserted from inputs)
NCORES = 8
EPS = 1e-12


# ----------------------------------------------------------------- host prep
def _prep(z, pos, edge_index, batch, batch_size, emb, centers, gamma):
    N = pos.shape[0]
    E = edge_index.shape[1]
    M = int(batch_size)
    R = centers.shape[0]

    batch = np.asarray(batch).astype(np.int64)
    row = np.asarray(edge_index[0]).astype(np.int64)
    col = np.asarray(edge_index[1]).astype(np.int64)
    pos = np.asarray(pos, dtype=np.float32)

    mol_bounds = np.array([round(c * M / NCORES) for c in range(NCORES + 1)])
    a_bounds = np.searchsorted(batch, mol_bounds)
    n_c = a_bounds[1:] - a_bounds[:-1]
    NPAD = int(np.ceil(max(n_c) / 128) * 128)
    NW = NPAD // 128
    m_c = mol_bounds[1:] - mol_bounds[:-1]
    assert max(m_c) <= 128

    # edge geometry (host, f32 to match the jax reference numerics)
    diff = pos[col] - pos[row]                                  # (E,3)
    dist = np.sqrt(np.sum(diff * diff, axis=1) + EPS, dtype=np.float32)
    dirs = diff / (dist[:, None] + np.float32(1e-8))            # (E,3)
    rbf = np.exp(-(np.asarray(gamma, np.float32)[None, :]
                   * (dist[:, None] - np.asarray(centers, np.float32)[None, :]) ** 2),
                 dtype=np.float32)                              # (E,R)

    core_of_atom = np.searchsorted(a_bounds, np.arange(N), side="right") - 1
    ecore = core_of_atom[row]
    col_core = core_of_atom[col]
    tab_row_of_edge = col_core * NPAD + (col - a_bounds[col_core])

    # per (core, window) edge lists, padded to a shared tile count
    per_core = []
    counts = np.zeros((NCORES, NW), np.int64)
    for c in range(NCORES):
        m = ecore == c
        rl = row[m] - a_bounds[c]
        order = np.argsort(rl, kind="stable")
        idx = np.nonzero(m)[0][order]
        rl = rl[order]
        w = rl // 128
        per_core.append((idx, rl, w))
        cw = np.bincount(w, minlength=NW)
        counts[c, :] = cw
    tiles_per_window = [int(np.ceil(counts[:, w].max() / 128)) for w in range(NW)]
    tiles_per_window = [max(t, 1) for t in tiles_per_window]
    T = sum(tiles_per_window)

    # packed per-tile arrays
    s0 = np.asarray(emb, np.float32)[np.asarray(z).astype(np.int64)]   # (N,F)

    colidx = np.zeros((NCORES, 128, T), np.int32)
    dirs_p = np.zeros((NCORES, 128, 3 * T), np.float32)
    rbf_p = np.zeros((NCORES, R + 1, 128 * T), np.float16)
    B_p = np.zeros((NCORES, 128, 128 * T), np.float16)
    for c in range(NCORES):
        idx, rl, w = per_core[c]
        colidx[c, :, :] = c * NPAD  # pad: row 0 of own shard
        tbase = 0
        for wi in range(NW):
            sel = w == wi
            ei = idx[sel]
            rli = rl[sel]
            k = len(ei)
            # positions: p = j % 128, tile = tbase + j // 128
            j = np.arange(k)
            tt = tbase + j // 128
            pp = j % 128
            colidx[c, pp, tt] = tab_row_of_edge[ei]
            for kk in range(3):
                dirs_p[c, pp, tt * 3 + kk] = dirs[ei, kk]
            rbf_p[c, :R, tt * 128 + pp] = rbf[ei].astype(np.float16)
            B_p[c, pp, tt * 128 + (rli - wi * 128)] = 1.0
            tbase += tiles_per_window[wi]
        rbf_p[c, R, :] = 1.0  # bias ones row

    # molecule one-hot per atom-window
    Bmol = np.zeros((NCORES, 128, 128 * NW), np.float16)
    for c in range(NCORES):
        a = np.arange(a_bounds[c], a_bounds[c + 1])
        al = a - a_bounds[c]
        ml = batch[a] - mol_bounds[c]
        Bmol[c, al % 128, (al // 128) * 128 + ml] = 1.0

    # initial state
    sT0 = np.zeros((NCORES, 128, NPAD), np.float32)
    tab0 = np.zeros((NCORES * NPAD, 4 * F), np.float16)
    for c in range(NCORES):
        sc = s0[a_bounds[c]:a_bounds[c + 1]]
        sT0[c, :, :sc.shape[0]] = sc.T
        tab0[c * NPAD:c * NPAD + sc.shape[0], :F] = sc.astype(np.float16)

    meta = dict(N=N, E=E, M=M, R=R, NPAD=NPAD, NW=NW, T=T,
                tiles_per_window=tiles_per_window,
                mol_bounds=mol_bounds, a_bounds=a_bounds, m_c=m_c)
    return meta, colidx, dirs_p, rbf_p, B_p, Bmol, sT0, tab0


def _prep_weights(meta, fw1, fb1, fw2, fb2, uw1, ub1, uw2, ub2,
                  ew1, eb1, ew2, eb2, dipole_w, nac_w,
                  aw1, ab1, aw2, ab2, yw1, yb1, yw2, yb2):
    f16 = np.float16
    L = fw1.shape[0]
    R = meta["R"]
    S = ew1.shape[0]
    P = nac_w.shape[0]
    w = {}
    w["fw1aug"] = np.concatenate(
        [np.asarray(fw1, np.float32), np.asarray(fb1, np.float32)[:, None, :]],
        axis=1).astype(f16)                                   # (L, R+1, F)
    w["fw2"] = np.asarray(fw2, np.float32).astype(f16)        # (L, F, 3F)
    w["fb2"] = np.asarray(fb2, np.float32)[:, None, :].astype(f16)  # (L,1,3F)
    w["fb2_nz"] = bool(np.any(np.asarray(fb2)))
    w["uw1c"] = np.asarray(uw1, np.float32).reshape(L, 3, F, F).astype(f16)
    w["ub1"] = np.asarray(ub1, np.float32).reshape(L, F, 1)
    w["uw2"] = np.asarray(uw2, np.float32).astype(f16)        # (L, F, 3F)
    w["ub2"] = np.asarray(ub2, np.float32).reshape(L, 3, F, 1)
    w["ub2_nz"] = bool(np.any(np.asarray(ub2)))
    w["ew1"] = np.asarray(ew1, np.float32).astype(f16)        # (S, F, F)
    w["eb1"] = np.asarray(eb1, np.float32).reshape(S, F, 1)
    w["ew2"] = np.asarray(ew2, np.float32).astype(f16)        # (S, F, 1)
    w["eb2"] = np.asarray(eb2, np.float32).reshape(S, 1, 1)
    w["wdn"] = np.concatenate(
        [np.asarray(dipole_w, np.float32)[:, None],
         np.asarray(nac_w, np.float32).T], axis=1)            # (F, 1+P)
    w["aw1aug"] = np.concatenate(
        [np.asarray(aw1, np.float32), np.asarray(ab1, np.float32)[None, :]],
        axis=0).astype(f16)                                   # (S-1+1, 64)
    w["aw2"] = np.asarray(aw2, np.float32).astype(f16)        # (64, 1)
    w["ab2"] = np.asarray(ab2, np.float32).reshape(1, 1)
    w["yw1aug"] = np.concatenate(
        [np.asarray(yw1, np.float32), np.asarray(yb1, np.float32)[None, :]],
        axis=0).astype(f16)                                   # (S-1+P+1, 128)
    w["yw2"] = np.asarray(yw2, np.float32).astype(f16)        # (128, 1)
    w["yb2"] = np.asarray(yb2, np.float32).reshape(1, 1)
    w["L"], w["S"], w["P"] = L, S, P
    return w


# ------------------------------------------------------------- device kernel
def _build(meta, w):
    from concourse import bacc, bass, mybir
    import concourse.tile as tile
    from concourse.masks import make_identity

    f16, f32, i32 = mybir.dt.float16, mybir.dt.float32, mybir.dt.int32
    AF = mybir.ActivationFunctionType
    OP = mybir.AluOpType

    NPAD, NW, T = meta["NPAD"], meta["NW"], meta["T"]
    TPW = meta["tiles_per_window"]
    L, S, P = w["L"], w["S"], w["P"]
    R = meta["R"]
    NG = (NPAD + 511) // 512
    TAB = NCORES * NPAD
    TWMAX = max(TPW)

    nc = bacc.Bacc("TRN2", target_bir_lowering=False)
    DI = lambda n, s, d: nc.dram_tensor(n, s, d, kind="ExternalInput")
    tab0 = DI("tab0", [TAB, 4 * F], f16)
    sT0 = DI("sT0", [128, NPAD], f32)
    rbfT = DI("rbfT", [R + 1, 128 * T], f16)
    dirsI = DI("dirs", [128, 3 * T], f32)
    colidx = DI("colidx", [128, T], i32)
    BblkI = DI("Bblk", [128, 128 * T], f16)
    BmolI = DI("Bmol", [128, 128 * NW], f16)
    wfw1 = DI("wfw1", [L, R + 1, F], f16)
    wfw2 = DI("wfw2", [L, F, 3 * F], f16)
    wfb2 = DI("wfb2", [L, 1, 3 * F], f16)
    wuw1 = DI("wuw1", [L, 3, F, F], f16)
    wub1 = DI("wub1", [L, F, 1], f32)
    wuw2 = DI("wuw2", [L, F, 3 * F], f16)
    wub2 = DI("wub2", [L, 3, F, 1], f32)
    wew1 = DI("wew1", [S, F, F], f16)
    web1 = DI("web1", [S, F, 1], f32)
    wew2 = DI("wew2", [S, F, 1], f16)
    web2 = DI("web2", [S, 1, 1], f32)
    wdn = DI("wdn", [F, 1 + P], f32)
    waw1 = DI("waw1", [S, 64], f16)           # S-1+1 = S rows
    waw2 = DI("waw2", [64, 1], f16)
    wab2 = DI("wab2", [1, 1], f32)
    wyw1 = DI("wyw1", [S + P, 128], f16)      # S-1+P+1 rows
    wyw2 = DI("wyw2", [128, 1], f16)
    wyb2 = DI("wyb2", [1, 1], f32)

    DO = lambda n, s: nc.dram_tensor(n, s, f32, kind="ExternalOutput")
    o_energy = DO("o_energy", [128, S])
    o_dipole = DO("o_dipole", [128, 3])
    o_nac = DO("o_nac", [128, P, 3])
    o_lam = DO("o_lam", [1, 128])
    o_phiy = DO("o_phiy", [1, 128])

    inb = nc.dram_tensor("inb", [NPAD, 4 * F], f16)
    tbl = nc.dram_tensor("tbl", [TAB, 4 * F], f16, addr_space="Shared")

    from contextlib import ExitStack
    with tile.TileContext(nc) as tc, ExitStack() as ctx:
        res = ctx.enter_context(tc.tile_pool(name="res", bufs=1))
        wk3 = ctx.enter_context(tc.tile_pool(name="wk3", bufs=3))
        wk2 = ctx.enter_context(tc.tile_pool(name="wk2", bufs=2))
        pm = ctx.enter_context(tc.tile_pool(name="pm", bufs=2, space="PSUM"))
        ph = ctx.enter_context(tc.tile_pool(name="ph", bufs=2, space="PSUM"))
        pp = ctx.enter_context(tc.tile_pool(name="pp", bufs=2, space="PSUM"))
        pt = ctx.enter_context(tc.tile_pool(name="pt", bufs=2, space="PSUM"))

        # ---- resident loads
        def load(pool, src, shape, dtype, tag):
            t_ = pool.tile(shape, dtype, tag=tag, name=tag)
            nc.sync.dma_start(out=t_[:], in_=src)
            return t_

        ident16 = res.tile([128, 128], f16, tag="id16")
        make_identity(nc, ident16)
        ident32 = res.tile([128, 128], f32, tag="id32")
        make_identity(nc, ident32)

        sT = load(res, sT0[:], [128, NPAD], f32, "sT")
        s16 = res.tile([128, NPAD], f16, tag="s16")
        nc.scalar.copy(out=s16[:], in_=sT[:])
        vT = []
        for k in range(3):
            v_ = res.tile([128, NPAD], f32, tag=f"vT{k}", name=f"vT{k}")
            nc.vector.memset(v_[:], 0.0)
            vT.append(v_)
        dirss = load(res, dirsI[:], [128, 3 * T], f32, "dirss")
        cidx = load(res, colidx[:], [128, T], i32, "cidx")
        Bmolsb = load(res, BmolI[:], [128, 128 * NW], f16, "Bmolsb")
        fw1s = res.tile([R + 1, L * F], f16, tag="fw1s")
        fw2s = res.tile([128, L * 3 * F], f16, tag="fw2s")
        fb2s = res.tile([1, L * 3 * F], f16, tag="fb2s")
        uw1s = res.tile([128, L * 3 * F], f16, tag="uw1s")
        ub1s = res.tile([128, L], f32, tag="ub1s")
        uw2s = res.tile([128, L * 3 * F], f16, tag="uw2s")
        ub2s = res.tile([128, L * 3], f32, tag="ub2s")
        ew1s = res.tile([128, S * F], f16, tag="ew1s")
        eb1s = res.tile([128, S], f32, tag="eb1s")
        ew2s = res.tile([128, S], f16, tag="ew2s")
        eb2s = res.tile([1, S], f32, tag="eb2s")
        for l in range(L):
            nc.sync.dma_start(out=fw1s[:, l * F:(l + 1) * F], in_=wfw1[l])
            nc.sync.dma_start(out=fw2s[:, l * 3 * F:(l + 1) * 3 * F], in_=wfw2[l])
            nc.sync.dma_start(out=fb2s[:, l * 3 * F:(l + 1) * 3 * F], in_=wfb2[l])
            nc.sync.dma_start(out=uw2s[:, l * 3 * F:(l + 1) * 3 * F], in_=wuw2[l])
            nc.sync.dma_start(out=ub1s[:, l:l + 1], in_=wub1[l])
            for j in range(3):
                nc.sync.dma_start(out=uw1s[:, (l * 3 + j) * F:(l * 3 + j + 1) * F],
                                  in_=wuw1[l, j])
                nc.sync.dma_start(out=ub2s[:, l * 3 + j:l * 3 + j + 1], in_=wub2[l, j])
        for s in range(S):
            nc.sync.dma_start(out=ew1s[:, s * F:(s + 1) * F], in_=wew1[s])
            nc.sync.dma_start(out=eb1s[:, s:s + 1], in_=web1[s])
            nc.sync.dma_start(out=ew2s[:, s:s + 1], in_=wew2[s])
            nc.sync.dma_start(out=eb2s[:, s:s + 1], in_=web2[s])
        wdns = load(res, wdn[:], [F, 1 + P], f32, "wdns")
        aw1s = load(res, waw1[:], [S, 64], f16, "aw1s")
        aw2s = load(res, waw2[:], [64, 1], f16, "aw2s")
        ab2s = load(res, wab2[:], [1, 1], f32, "ab2s")
        yw1s = load(res, wyw1[:], [S + P, 128], f16, "yw1s")
        yw2s = load(res, wyw2[:], [128, 1], f16, "yw2s")
        yb2s = load(res, wyb2[:], [1, 1], f32, "yb2s")
        ones16 = res.tile([1, 128], f16, tag="ones16")
        nc.vector.memset(ones16[:], 1.0)
        epsb = res.tile([128, 1], f32, tag="epsb")
        nc.vector.memset(epsb[:], EPS)

        msT = res.tile([128, NPAD], f16, tag="msT")
        mvT = [res.tile([128, NPAD], f16, tag=f"mvT{k}", name=f"mvT{k}") for k in range(3)]
        vn16 = res.tile([128, NPAD], f16, tag="vn16")

        # ------------------------------------------------ layer loop
        for l in range(L):
            src_tab = tab0 if l == 0 else tbl
            tptr = 0
            h16 = None
            for wi in range(NW):
                Tw = TPW[wi]
                mps = pm.tile([128, 512], f32, tag="m")
                Bw = wk2.tile([128, TWMAX * 128], f16, tag="Bw")
                nc.sync.dma_start(
                    out=Bw[:, :Tw * 128],
                    in_=BblkI[:, tptr * 128:(tptr + Tw) * 128])
                for ti in range(Tw):
                    t = tptr + ti
                    gi = t % 4
                    if gi == 0:
                        ncols = min(4, T - t) * 128
                        rbfg = wk3.tile([R + 1, 512], f16, tag="rbfg")
                        nc.sync.dma_start(out=rbfg[:, :ncols],
                                          in_=rbfT[:, t * 128:t * 128 + ncols])
                        hp = ph.tile([128, 512], f32, tag="h")
                        nc.tensor.matmul(
                            out=hp[:, :ncols],
                            lhsT=fw1s[:, l * F:(l + 1) * F],
                            rhs=rbfg[:, :ncols],
                            start=True, stop=True)
                        h16 = wk2.tile([128, 512], f16, tag="h16")
                        nc.scalar.activation(out=h16[:, :ncols], in_=hp[:, :ncols],
                                             func=AF.Silu)
                    # gather
                    gt = wk3.tile([128, 4 * F], f16, tag="gt")
                    nc.gpsimd.indirect_dma_start(
                        out=gt[:], out_offset=None, in_=src_tab[:],
                        in_offset=bass.IndirectOffsetOnAxis(
                            ap=cidx[:, t:t + 1], axis=0))
                    # phi
                    pphi = pp.tile([128, 3 * F], f32, tag="phi")
                    nc.tensor.matmul(out=pphi[:], lhsT=h16[:, gi * 128:(gi + 1) * 128],
                                     rhs=fw2s[:, l * 3 * F:(l + 1) * 3 * F], start=True,
                                     stop=not w["fb2_nz"])
                    if w["fb2_nz"]:
                        nc.tensor.matmul(out=pphi[:], lhsT=ones16[:],
                                         rhs=fb2s[:, l * 3 * F:(l + 1) * 3 * F], start=False, stop=True)
                    # messages (DVE), fp16 out
                    msg = wk3.tile([128, 4 * F], f16, tag="msg")
                    tmp = wk2.tile([128, F], f16, tag="tmp")
                    mv1 = wk2.tile([128, F], f16, tag="mv1")
                    nc.vector.tensor_tensor(out=msg[:, 0:F], in0=pphi[:, 0:F],
                                            in1=gt[:, 0:F], op=OP.mult)
                    nc.vector.tensor_tensor(out=tmp[:], in0=pphi[:, 2 * F:3 * F],
                                            in1=gt[:, 0:F], op=OP.mult)
                    for k in range(3):
                        nc.vector.tensor_tensor(
                            out=mv1[:], in0=pphi[:, F:2 * F],
                            in1=gt[:, (1 + k) * F:(2 + k) * F], op=OP.mult)
                        nc.vector.scalar_tensor_tensor(
                            out=msg[:, (1 + k) * F:(2 + k) * F],
                            in0=tmp[:], scalar=dirss[:, 3 * t + k:3 * t + k + 1],
                            in1=mv1[:], op0=OP.mult, op1=OP.add)
                    # segment sum
                    nc.tensor.matmul(out=mps[:],
                                     lhsT=Bw[:, (ti * 128):(ti + 1) * 128],
                                     rhs=msg[:], start=(ti == 0),
                                     stop=(ti == Tw - 1))
                # drain window wi
                m16 = wk2.tile([128, 512], f16, tag="m16")
                nc.scalar.copy(out=m16[:], in_=mps[:])
                for j in range(4):
                    trp = pt.tile([128, 128], f16, tag="tr")
                    nc.tensor.transpose(out=trp[:], in_=m16[:, j * 128:(j + 1) * 128],
                                        identity=ident16[:])
                    dst = msT if j == 0 else mvT[j - 1]
                    nc.scalar.copy(out=dst[:, wi * 128:(wi + 1) * 128], in_=trp[:])
                tptr += Tw

            # v_norm (groupwise to save SBUF)
            for g in range(NG):
                sl = slice(g * 512, min((g + 1) * 512, NPAD))
                vng = wk2.tile([128, 512], f32, tag="tva")
                vnh = wk2.tile([128, 512], f32, tag="tvb")
                nw_ = sl.stop - sl.start
                nc.vector.tensor_tensor(out=vng[:, :nw_], in0=mvT[0][:, sl],
                                        in1=mvT[0][:, sl], op=OP.mult)
                for k in (1, 2):
                    nc.vector.tensor_tensor(out=vnh[:, :nw_], in0=mvT[k][:, sl],
                                            in1=mvT[k][:, sl], op=OP.mult)
                    nc.vector.tensor_tensor(out=vng[:, :nw_], in0=vng[:, :nw_],
                                            in1=vnh[:, :nw_], op=OP.add)
                nc.scalar.activation(out=vn16[:, sl], in_=vng[:, :nw_],
                                     func=AF.Sqrt, bias=epsb[:])

            # update MLP per 512-atom group
            for g in range(NG):
                lo = g * 512
                ncols = min(512, NPAD - lo)
                sl = slice(lo, lo + ncols)
                up1 = ph.tile([128, 512], f32, tag="h")
                for j, src in enumerate((s16, msT, vn16)):
                    nc.tensor.matmul(out=up1[:, :ncols], lhsT=uw1s[:, (l * 3 + j) * F:(l * 3 + j + 1) * F],
                                     rhs=src[:, sl], start=(j == 0), stop=(j == 2))
                u116 = wk2.tile([128, 512], f16, tag="h16")
                nc.scalar.activation(out=u116[:, :ncols], in_=up1[:, :ncols],
                                     func=AF.Silu, bias=ub1s[:, l:l + 1])
                pj = []
                for j in range(3):
                    if j == 0:
                        pj_ = pp.tile([128, 512], f32, tag="phi", name="pj0")
                    elif j == 1:
                        pj_ = pm.tile([128, 512], f32, tag="m", name="pj1")
                    else:
                        pj_ = pt.tile([128, 512], f32, tag="tr", name="pj2")
                    nc.tensor.matmul(out=pj_[:, :ncols],
                                     lhsT=uw2s[:, (l * 3 + j) * F:(l * 3 + j + 1) * F],
                                     rhs=u116[:, :ncols], start=True, stop=True)
                    if w["ub2_nz"]:
                        nc.vector.tensor_scalar_add(out=pj_[:, :ncols],
                                                    in0=pj_[:, :ncols],
                                                    scalar1=ub2s[:, l * 3 + j:l * 3 + j + 1])
                    pj.append(pj_)
                nc.vector.tensor_tensor(out=sT[:, sl], in0=sT[:, sl],
                                        in1=pj[0][:, :ncols], op=OP.add)
                nc.scalar.copy(out=s16[:, sl], in_=sT[:, sl])
                tva = wk2.tile([128, 512], f32, tag="tva")
                tvb = wk2.tile([128, 512], f32, tag="tvb")
                for k in range(3):
                    nc.vector.tensor_tensor(out=tvb[:, :ncols], in0=pj[2][:, :ncols],
                                            in1=mvT[k][:, sl], op=OP.mult)
                    nc.vector.tensor_tensor(out=tva[:, :ncols], in0=pj[1][:, :ncols],
                                            in1=vT[k][:, sl], op=OP.mult)
                    nc.vector.tensor_tensor(out=vT[k][:, sl], in0=tva[:, :ncols],
                                            in1=tvb[:, :ncols], op=OP.add)

            # table writeback + allgather (not needed after last layer)
            if l < L - 1:
                for wi in range(NW):
                    sl = slice(wi * 128, (wi + 1) * 128)
                    srcw = wk2.tile([128, 4 * F], f16, tag="srcw")
                    trp = pt.tile([128, 128], f16, tag="tr")
                    nc.tensor.transpose(out=trp[:], in_=s16[:, sl], identity=ident16[:])
                    nc.scalar.copy(out=srcw[:, 0:F], in_=trp[:])
                    for k in range(3):
                        trp32 = pt.tile([128, 128], f32, tag="tr")
                        nc.tensor.transpose(out=trp32[:], in_=vT[k][:, sl],
                                            identity=ident32[:])
                        nc.scalar.copy(out=srcw[:, (1 + k) * F:(2 + k) * F],
                                       in_=trp32[:])
                    nc.sync.dma_start(out=inb[sl, :], in_=srcw[:])
                nc.gpsimd.collective_compute(
                    "AllGather", mybir.AluOpType.bypass,
                    replica_groups=[list(range(NCORES))],
                    ins=[inb.ap().opt()], outs=[tbl.ap().opt()])

        # ------------------------------------------------ heads
        eat = vn16   # reuse (dead after last update)
        dnr = msT
        nc.vector.memset(eat[:], 0.0)
        nc.vector.memset(dnr[:], 0.0)
        for g in range(NG):
            lo = g * 512
            ncols = min(512, NPAD - lo)
            sl = slice(lo, lo + ncols)
            for s in range(S):
                hp = ph.tile([128, 512], f32, tag="h")
                nc.tensor.matmul(out=hp[:, :ncols], lhsT=ew1s[:, s * F:(s + 1) * F],
                                 rhs=s16[:, sl], start=True, stop=True)
                h16_ = wk2.tile([128, 512], f16, tag="h16")
                nc.scalar.activation(out=h16_[:, :ncols], in_=hp[:, :ncols],
                                     func=AF.Silu, bias=eb1s[:, s:s + 1])
                ep = pp.tile([1, 512], f32, tag="phi")
                nc.tensor.matmul(out=ep[:, :ncols], lhsT=ew2s[:, s:s + 1],
                                 rhs=h16_[:, :ncols], start=True, stop=True)
                est = wk2.tile([1, 512], f16, tag="est")
                nc.scalar.activation(out=est[:, :ncols], in_=ep[:, :ncols],
                                     func=AF.Identity, bias=eb2s[0:1, s:s + 1])
                nc.sync.dma_start(out=eat[s:s + 1, sl], in_=est[:, :ncols])
            for k in range(3):
                dp = pp.tile([1 + P, 512], f32, tag="phi")
                nc.tensor.matmul(out=dp[:, :ncols], lhsT=wdns[:],
                                 rhs=vT[k][:, sl], start=True, stop=True)
                dst4 = wk2.tile([1 + P, 512], f16, tag="dst4")
                nc.scalar.copy(out=dst4[:, :ncols], in_=dp[:, :ncols])
                nc.sync.dma_start(out=dnr[S + k * 4:S + (k + 1) * 4, sl],
                                  in_=dst4[:, :ncols])

        molp = pm.tile([128, 16], f32, tag="m")
        for wi in range(NW):
            sl = slice(wi * 128, (wi + 1) * 128)
            hv = wk2.tile([128, 16], f16, tag="hv")
            trp = pt.tile([128, 128], f16, tag="tr")
            nc.tensor.transpose(out=trp[:], in_=eat[:, sl], identity=ident16[:])
            nc.scalar.copy(out=hv[:, 0:S], in_=trp[:, 0:S])
            trp2 = pt.tile([128, 128], f16, tag="tr")
            nc.tensor.transpose(out=trp2[:], in_=dnr[:, sl], identity=ident16[:])
            nc.scalar.copy(out=hv[:, S:S + 12], in_=trp2[:, S:S + 12])
            nc.tensor.matmul(out=molp[:], lhsT=Bmolsb[:, wi * 128:(wi + 1) * 128],
                             rhs=hv[:], start=(wi == 0), stop=(wi == NW - 1))

        # molp cols: 0..S-1 energies; S+k*4 = dip_k; S+k*4+1+p = nac[p,k]
        molsb = wk2.tile([128, S + 12], f32, tag="molsb")
        nc.scalar.copy(out=molsb[:], in_=molp[:, 0:S + 12])
        nc.sync.dma_start(out=o_energy[:], in_=molsb[:, 0:S])
        dipap = molsb[:, S:S + 12].rearrange("p (k j) -> p k j", k=3)[:, :, 0]
        nc.sync.dma_start(out=o_dipole[:], in_=dipap)
        # nac out (128, P, 3): [p_idx, k] <- molsb col S + k*4 + 1 + p
        nacap = molsb[:, S:S + 12].rearrange("q (k j) -> q j k", k=3)[:, 1:1 + P, :]
        nacsb = wk2.tile([128, P, 3], f32, tag="nacsb")
        nc.vector.tensor_copy(out=nacsb[:], in_=nacap)
        nc.sync.dma_start(out=o_nac[:], in_=nacsb[:])

        # gaps (128, S-1) f32
        gaps = wk2.tile([128, S - 1], f32, tag="gaps")
        nc.vector.tensor_scalar(out=gaps[:], in0=molsb[:, 1:S],
                                scalar1=molsb[:, 0:1], scalar2=None,
                                op0=mybir.AluOpType.subtract)
        # nac_mag (128, P)
        nsq = wk2.tile([128, P, 3], f32, tag="nsq")
        nc.scalar.activation(out=nsq[:], in_=nacsb[:], func=AF.Square)
        nm = wk2.tile([128, P], f32, tag="nm")
        nc.vector.tensor_reduce(out=nm[:], in_=nsq[:], axis=mybir.AxisListType.X,
                                op=mybir.AluOpType.add)
        nmr = wk2.tile([128, P], f32, tag="nmr")
        nc.scalar.activation(out=nmr[:], in_=nm[:], func=AF.Sqrt, bias=epsb[:])

        # catq (128, S-1+P) f16 -> transpose -> (S-1+P, 128) + ones row
        catq = wk2.tile([128, 128], f16, tag="catq")
        nc.vector.memset(catq[:], 0.0)
        nc.vector.tensor_copy(out=catq[:, 0:S - 1], in_=gaps[:])
        nc.vector.tensor_copy(out=catq[:, S - 1:S - 1 + P], in_=nmr[:])
        trq = pt.tile([128, 128], f16, tag="tr")
        nc.tensor.transpose(out=trq[:], in_=catq[:], identity=ident16[:])
        catT = wk2.tile([S + P, 128], f16, tag="catT")
        nc.vector.tensor_copy(out=catT[0:S - 1 + P, :], in_=trq[0:S - 1 + P, :])
        nc.sync.dma_start(out=catT[S - 1 + P:S + P, :], in_=ones16[:])

        # lam = silu(gaps@aw1+ab1)@aw2 + ab2
        lp1 = ph.tile([64, 128], f32, tag="h")
        gin = catT[0:S - 1, :]  # (S-1, 128)
        aug = wk2.tile([S, 128], f16, tag="aug")
        nc.vector.tensor_copy(out=aug[0:S - 1, :], in_=gin)
        nc.sync.dma_start(out=aug[S - 1:S, :], in_=ones16[:])
        nc.tensor.matmul(out=lp1[:], lhsT=aw1s[:], rhs=aug[:], start=True, stop=True)
        l116 = wk2.tile([64, 128], f16, tag="l116")
        nc.scalar.activation(out=l116[:], in_=lp1[:], func=AF.Silu)
        lp2 = pp.tile([1, 128], f32, tag="phi")
        nc.tensor.matmul(out=lp2[:], lhsT=aw2s[:], rhs=l116[:], start=True, stop=True)
        lamr = wk2.tile([1, 128], f32, tag="lamr")
        nc.scalar.activation(out=lamr[:], in_=lp2[:], func=AF.Identity, bias=ab2s[:])
        nc.sync.dma_start(out=o_lam[:], in_=lamr[:])

        # phi_y = sigmoid(silu(cat@yw1+yb1)@yw2+yb2)
        yp1 = ph.tile([128, 128], f32, tag="h")
        nc.tensor.matmul(out=yp1[:], lhsT=yw1s[:], rhs=catT[:], start=True, stop=True)
        y116 = wk2.tile([128, 128], f16, tag="y116")
        nc.scalar.activation(out=y116[:], in_=yp1[:], func=AF.Silu)
        yp2 = pp.tile([1, 128], f32, tag="phi")
        nc.tensor.matmul(out=yp2[:], lhsT=yw2s[:], rhs=y116[:], start=True, stop=True)
        phiyr = wk2.tile([1, 128], f32, tag="phiyr")
        nc.scalar.activation(out=phiyr[:], in_=yp2[:], func=AF.Sigmoid, bias=yb2s[:])
        nc.sync.dma_start(out=o_phiy[:], in_=phiyr[:])

    nc.finalize()
    return nc


def _install_ntff_hook():
    """Optional: register the NTFF profile hook so trace=True works under axon."""
    try:
        import sys, types
        import antenv
        if "antenv.axon_hooks" in sys.modules:
            return
        mod = types.ModuleType("antenv.axon_hooks")
        _h = [None]
        mod.set_axon_ntff_profile_hook = lambda h: _h.__setitem__(0, h)
        mod.get_axon_ntff_profile_hook = lambda: _h[0]
        sys.modules["antenv.axon_hooks"] = mod
        antenv.axon_hooks = mod
        from trn_agent_boot.trn_boot import _ntff_profile_via_ctypes
        mod.set_axon_ntff_profile_hook(
            _ntff_profile_via_ctypes("/opt/axon/libaxon_pjrt.so"))
    except Exception:
        pass


# ------------------------------------------------------------------ kernel()
def kernel(**inputs):
    meta, colidx, dirs_p, rbf_p, B_p, Bmol, sT0, tab0 = _prep(
        inputs["z"], inputs["pos"], inputs["edge_index"], inputs["batch"],
        inputs["batch_size"], inputs["emb"], inputs["centers"], inputs["gamma"])
    w = _prep_weights(meta, *[inputs[k] for k in
        ("fw1", "fb1", "fw2", "fb2", "uw1", "ub1", "uw2", "ub2",
         "ew1", "eb1", "ew2", "eb2", "dipole_w", "nac_w",
         "aw1", "ab1", "aw2", "ab2", "yw1", "yb1", "yw2", "yb2")])
    nc = _build(meta, w)

    shared = dict(
        tab0=tab0, wdn=w["wdn"].astype(np.float32),
        wfw1=w["fw1aug"], wfw2=w["fw2"], wfb2=w["fb2"],
        wuw1=w["uw1c"], wub1=w["ub1"], wuw2=w["uw2"], wub2=w["ub2"],
        wew1=w["ew1"], web1=w["eb1"], wew2=w["ew2"], web2=w["eb2"],
        waw1=w["aw1aug"], waw2=w["aw2"], wab2=w["ab2"],
        wyw1=w["yw1aug"], wyw2=w["yw2"], wyb2=w["yb2"])
    in_maps = []
    for c in range(NCORES):
        m = dict(shared)
        m.update(sT0=sT0[c], rbfT=rbf_p[c], dirs=dirs_p[c],
                 colidx=colidx[c], Bblk=B_p[c], Bmol=Bmol[c])
        in_maps.append(m)

    from concourse.bass_utils import run_bass_kernel_spmd
    import os
    trace = os.environ.get("GNN_TRACE") == "1"
    if trace:
        _install_ntff_hook()
    res = run_bass_kernel_spmd(nc, in_maps, list(range(NCORES)), trace=trace)
    if trace and res.exec_time_ns:
        print(f"HW exec time: {res.exec_time_ns} ns")
        kernel.last_exec_ns = res.exec_time_ns

    M = meta["M"]
    S, P = w["S"], w["P"]
    mb = meta["mol_bounds"]
    energies = np.zeros((M, S), np.float32)
    dipoles = np.zeros((M, 3), np.float32)
    nac = np.zeros((M, P, 3), np.float32)
    lam = np.zeros((M, 1), np.float32)
    phiy = np.zeros((M, 1), np.float32)
    for c in range(NCORES):
        mc = mb[c + 1] - mb[c]
        r = res.results[c]
        energies[mb[c]:mb[c + 1]] = r["o_energy"][:mc]
        dipoles[mb[c]:mb[c + 1]] = r["o_dipole"][:mc]
        nac[mb[c]:mb[c + 1]] = r["o_nac"][:mc]
        lam[mb[c]:mb[c + 1]] = r["o_lam"][0, :mc, None]
        phiy[mb[c]:mb[c + 1]] = r["o_phiy"][0, :mc, None]
    return energies, dipoles, nac, lam, phiy
